# revision 57
# baseline (speedup 1.0000x reference)
"""ExpertNet (moe_routing) Trainium2 Bass kernel — v2.

Data-parallel over 8 NeuronCores: batch N=32768 split into 8 shards of 4096.
All parameters replicated.  Per-core pipeline (activations transposed
[feature, sample], moving free dim NB=512):

  X --(fp8 DoubleRow PE, hi/lo compensated)--> h --(fp16 PE)--> z
    --(fp16 PE)--> dist --(DVE reciprocal)--> q' = 64/(1+dist)
    --(fp8-DR selector)--> q broadcast --(DVE)--> z*q
    --(fp16 PE, row-packed expert pairs)--> expert hidden
    --(ACT/DVE relu per cfg["epat"])--> eh --(fp16 PE accumulate)--> preds^T
    --(fused DVE scale by rqs, DVE transpose)--> preds --> DRAM

Cost-model: 187.2us vs 232.6us for the fp32r v1 baseline (PE-busy ~169us,
~90% occupancy).  Key points vs v1:
  * Encoder runs in fp8e4m3 DoubleRow perf mode (0.5 PE-cycles/column).
    X is split on the HOST into hi+lo fp8 (X ~ hi+lo to ~0.2%), enc_W into
    Whi+Wlo.  Three DR passes (Whi.Xhi 2-chunk, Whi.Xlo 2-chunk, Wlo.Xhi
    2-chunk) give a bf16-accurate encoder in 24 units/block vs 32.  fp8
    WITHOUT hi/lo compensation fails the 2e-2 gate (~3e-2 per source), and
    compensating an on-chip moving tensor costs extra elementwise ops, so
    DR is only used where the split is free (X: host; q: 2 tiny Pool ops).
  * Everything else runs fp16 (same 1 cyc/row PE rate as fp32r/bf16,
    ~11-bit mantissa keeps rel-err ~1.7e-3).  W1/W2 are input/output bound
    on the PE (eh elements / 128 per cycle) — 32+32 units is the floor.
  * dist is ONE fp8-DoubleRow matmul: group0 = 4z vs -2mu^T, group1 =
    16z^2 (ACT Square) vs 0.25; the (1+|mu|^2) bias rides row 64 (a 4.0
    ones-row, set once in two fixed zdist tiles) with hi/lo fp8
    compensation across the groups.  qr = recip(PSUM) = q_un/4; the x256
    rescale folds into the DVE q hi/lo split ops.  fp8 z here is safe:
    dist errors are ~0.2% relative and q is later normalized.
  * zt2 is the duplicated-column Wz product so one ACT copy yields both
    64-row copies for the zq path.
  * q broadcast to expert-pair partitions is one fp8-DoubleRow matmul per
    pair against a 0/1 selector, with q split hi+lo fp8 on DVE (exact to
    0.1%; walrus rejects tensor_scalar on Pool): 4 units/block vs 8.
  * rqs = 1/(64 sum q) broadcast via gpsimd.partition_broadcast (no PE);
    the 64-normalization of q cancels exactly against it in the tail stt.
  * b2 == 0 path skips the b2 matmul entirely (start=True on the first
    combine matmul zero-fills the 32-row preds PSUM).
  * back() is software-pipelined: W2(i) trails W1(i) by cfg["lag"] PE
    slots so the PE never waits on a relu; e2/zq for a pair are emitted
    cfg["eq"] slots early.  X DMAs are one contiguous transfer per block
    (host pre-blocks the layout), spread across SP and Pool DGE queues;
    gpsimd CANNOT touch PSUM (relus are ACT/DVE only).
"""

import numpy as np

N, D, H_ENC, NZ, KE, H_EXP, C = 32768, 1024, 512, 64, 16, 256, 10
NCORES = 8
NS = N // NCORES          # samples per core
NB = 512                  # samples per block (matmul moving free dim)
NBLK = NS // NB
NPAIR = KE // 2           # expert pairs (row-packed)
QS = 64.0                 # q scale (q' = QS * q_unnormalized)
SX = 16.0                 # X fp8 scale
SW = 512.0                # enc_W fp8 scale

_CACHE = {}
LAST_RESULTS = None


def _build(has_b1: bool, has_b2: bool, cfg: dict | None = None):
    defaults = dict(pbig=4, pmisc=1, pqb=2, ppred=1, hbufs=6, ehbufs=14,
                    zqbufs=2, xbufs=3, repeat=1, ahead=2, lag=3, eq=3,
                    epat="AADADAADADAADADA")
    cfg = {**defaults, **(cfg or {})}
    import concourse.bacc as bacc
    import concourse.mybir as mybir
    from concourse import tile

    F32 = mybir.dt.float32
    F16 = mybir.dt.float16
    F8 = mybir.dt.float8e4
    AF = mybir.ActivationFunctionType
    DR = mybir.MatmulPerfMode.DoubleRow
    W = NB

    nc = bacc.Bacc("TRN2", target_bir_lowering=False, debug=False,
                   num_devices=NCORES)

    # ---- I/O ----------------------------------------------------------
    XHI = nc.dram_tensor("XHI", [NBLK, 128, 8 * NB], F8, kind="ExternalInput")
    XLO = nc.dram_tensor("XLO", [NBLK, 128, 8 * NB], F8, kind="ExternalInput")
    WEH = nc.dram_tensor("WEH", [128, 4 * 4 * 2 * 128], F8, kind="ExternalInput")
    WEL = nc.dram_tensor("WEL", [128, 4 * 4 * 2 * 128], F8, kind="ExternalInput")
    BENC = nc.dram_tensor("BENC", [128, 4], F32, kind="ExternalInput")
    WZ = nc.dram_tensor("WZ", [128, 4 * 128], F16, kind="ExternalInput")
    BZ = nc.dram_tensor("BZ", [128, 1], F32, kind="ExternalInput")
    BZ4 = nc.dram_tensor("BZ4", [NZ, 1], F32, kind="ExternalInput")
    DDIST = nc.dram_tensor("DDIST", [NZ + 1, 2 * KE], F8, kind="ExternalInput")
    O16 = nc.dram_tensor("O16", [KE, 1], F16, kind="ExternalInput")
    E2P = nc.dram_tensor("E2P", [KE, NPAIR * 2 * 128], F8, kind="ExternalInput")
    W1P = nc.dram_tensor("W1P", [128, NPAIR * H_EXP], F16, kind="ExternalInput")
    W2C = nc.dram_tensor("W2C", [128, KE * 2 * 32], F16, kind="ExternalInput")
    if has_b2:
        B2Q = nc.dram_tensor("B2Q", [KE, 32], F16, kind="ExternalInput")
    if has_b1:
        B1C = nc.dram_tensor("B1C", [128, KE * 2], F32, kind="ExternalInput")
        E2S = nc.dram_tensor("E2S", [KE, KE * 2 * 128], F8, kind="ExternalInput")
    OUT = nc.dram_tensor("OUT", [NS, C], F32, kind="ExternalOutput")

    epat = cfg["epat"]

    with tile.TileContext(nc) as tc, nc.allow_low_precision(
        reason="fp16 tiles + hi/lo-compensated fp8; rel-err ~1e-3"
    ):
        with (
            tc.tile_pool(name="wpool", bufs=1) as wp,
            tc.tile_pool(name="xpool", bufs=cfg["xbufs"]) as xp,
            tc.tile_pool(name="hpool", bufs=cfg["hbufs"]) as hp,
            tc.tile_pool(name="zpool", bufs=cfg.get("zbufs", 2)) as zp,
            tc.tile_pool(name="qpool", bufs=cfg.get("qbufs", 2)) as qp,
            tc.tile_pool(name="zqpool", bufs=cfg["zqbufs"]) as zqp,
            tc.tile_pool(name="ehpool", bufs=cfg["ehbufs"]) as ehp,
            tc.tile_pool(name="trpool", bufs=3) as trp,
            tc.tile_pool(name="pbig", bufs=cfg["pbig"], space="PSUM") as pbig,
            tc.tile_pool(name="pmisc", bufs=cfg["pmisc"], space="PSUM") as pmisc,
            tc.tile_pool(name="pqb", bufs=cfg["pqb"], space="PSUM") as pqb,
            tc.tile_pool(name="ppred", bufs=cfg["ppred"], space="PSUM") as ppred,
        ):
            # ---- load weights once -----------------------------------
            def wload(dram, shape, dt):
                t = wp.tile(shape, dt, name=dram.name + "_sb")
                nc.sync.dma_start(t[:], dram[:])
                return t

            # block-0 X and the encoder weights are DMA'd in interleaved
            # dcp-slices so the first encoder matmul (needs only dcp=0 of
            # xhi+weh) starts as early as possible
            xhi0 = xp.tile([128, 8 * NB], F8, tag="xhi")
            xlo0 = xp.tile([128, 8 * NB], F8, tag="xlo")
            weh = wp.tile([128, 4096], F8, name="WEH_sb")
            wel = wp.tile([128, 4096], F8, name="WEL_sb")
            for dcp in range(4):
                xs = slice(2 * dcp * NB, (2 * dcp + 2) * NB)
                ws = slice(dcp * 1024, (dcp + 1) * 1024)
                nc.sync.dma_start(xhi0[:, xs], XHI[0, :, xs])
                nc.gpsimd.dma_start(weh[:, ws], WEH[:, ws])
            nc.gpsimd.dma_start(xlo0[:], XLO[0, :, :])
            nc.scalar.dma_start(wel[:], WEL[:])
            benc = wload(BENC, [128, 4], F32)
            wz = wload(WZ, [128, 512], F16)
            bz = wload(BZ, [128, 1], F32)
            bz4 = wload(BZ4, [NZ, 1], F32)
            ddist = wload(DDIST, [NZ + 1, 2 * KE], F8)
            o16 = wload(O16, [KE, 1], F16)
            # two fixed zdist tiles, alternated per block: row 64 (the 4.0
            # ones-row that carries the dist bias) is set once, not per block
            zdist_fix = []
            for zi in range(2):
                zd = wp.tile([NZ + 1, 2 * NB], F8, name=f"zdist{zi}")
                nc.gpsimd.memset(zd[NZ:NZ + 1, :], 4.0)
                zdist_fix.append(zd)
            pre_x = {0: (xhi0, xlo0)}
            for pb in (1, 2):
                xh = xp.tile([128, 8 * NB], F8, tag="xhi")
                xl = xp.tile([128, 8 * NB], F8, tag="xlo")
                nc.sync.dma_start(xh[:], XHI[pb, :, :])
                nc.gpsimd.dma_start(xl[:], XLO[pb, :, :])
                pre_x[pb] = (xh, xl)

            late = {}

            def wload_q(dram, shape, dt, eng):
                t = wp.tile(shape, dt, name=dram.name + "_sb")
                eng.dma_start(t[:], dram[:])
                return t

            def load_late_weights():
                # Pool/scalar DMA queues: don't compete with X on sync
                late["e2p"] = wload_q(E2P, [KE, NPAIR * 2 * 128], F8, nc.scalar)
                late["w1p"] = wload_q(W1P, [128, NPAIR * H_EXP], F16, nc.gpsimd)
                late["w2c"] = wload_q(W2C, [128, KE * 2 * 32], F16, nc.gpsimd)
                if has_b2:
                    late["b2q"] = wload_q(B2Q, [KE, 32], F16, nc.scalar)
                if has_b1:
                    late["b1c"] = wload_q(B1C, [128, KE * 2], F32, nc.scalar)
                    late["e2s"] = wload_q(E2S, [KE, KE * 2 * 128], F8, nc.scalar)

            def front(ib, pre=None):
                n0 = ib * NB
                if pre is not None:
                    xhi, xlo = pre
                else:
                    xhi = xp.tile([128, 8 * W], F8, tag="xhi")
                    xlo = xp.tile([128, 8 * W], F8, tag="xlo")
                    nc.sync.dma_start(xhi[:], XHI[ib, :, :])
                    nc.gpsimd.dma_start(xlo[:], XLO[ib, :, :])

                def xpair(t, dcp):
                    return t[:, 2 * dcp * W:(2 * dcp + 2) * W].rearrange(
                        "p (t n) -> p t n", t=2)

                def wpair(t, dcp, hc):
                    o = (dcp * 4 + hc) * 256
                    return t[:, o:o + 256].rearrange("p (t c) -> p t c", t=2)

                # encoder: h = relu((Whi+Wlo)^T (Xhi+Xlo) / (SX*SW) + b)
                # For the very first block, emit ALL hi*hi passes before the
                # cross passes: the first matmuls then depend only on the
                # earliest DMAs (xhi + weh), not on xlo/wel.
                def enc_pass(ph, hc, pi, first, last):
                    src_w = wel if pi == 2 else weh
                    src_x = xlo if pi == 1 else xhi
                    for dcp in range(4):
                        nc.tensor.matmul(
                            ph[:], wpair(src_w, dcp, hc), xpair(src_x, dcp),
                            perf_mode=DR, start=(first and dcp == 0),
                            stop=(last and dcp == 3))

                def enc_relu(ph, hc):
                    ht = hp.tile([128, W], F16, tag="ht")
                    nc.scalar.activation(ht[:], ph[:], AF.Relu,
                                         bias=benc[:, hc:hc + 1],
                                         scale=1.0 / (SX * SW))
                    return ht

                hts = []
                if ib == 0:
                    phs = [pbig.tile([128, W], F32, tag="pbig",
                                     name=f"ph{hc}")
                           for hc in range(4)]
                    for hc in range(4):
                        enc_pass(phs[hc], hc, 0, True, False)
                    for hc in range(4):
                        enc_pass(phs[hc], hc, 1, False, False)
                        enc_pass(phs[hc], hc, 2, False, True)
                        hts.append(enc_relu(phs[hc], hc))
                else:
                    for hc in range(4):
                        ph = pbig.tile([128, W], F32, tag="pbig")
                        enc_pass(ph, hc, 0, True, False)
                        enc_pass(ph, hc, 1, False, False)
                        enc_pass(ph, hc, 2, False, True)
                        hts.append(enc_relu(ph, hc))

                # z layer (Wz columns duplicated -> zt2 psum [128, W])
                pz = pmisc.tile([128, W], F32, tag="pmisc")
                for hc in range(4):
                    nc.tensor.matmul(pz[:], wz[:, hc * 128:(hc + 1) * 128],
                                     hts[hc][:], start=(hc == 0), stop=(hc == 3))
                zt2 = zp.tile([128, W], F16, tag="zt2")
                nc.scalar.activation(zt2[:], pz[:], AF.Identity, bias=bz[:])
                # fp8 dist operands: zdist cols 0:W = 4z, cols W:2W = 16z^2;
                # row 64 = 1.0 rides the (1+|mu|^2) bias (hi/lo compensated
                # across the two DoubleRow groups of DDIST)
                zdist = zdist_fix[ib % 2]
                nc.scalar.activation(zdist[0:NZ, 0:W], pz[0:NZ, :], AF.Identity,
                                     bias=bz4[:], scale=4.0)
                nc.scalar.activation(zdist[0:NZ, W:2 * W], pz[0:NZ, :], AF.Square,
                                     bias=bz4[:], scale=4.0)

                # pd = 4*(1+dist) in ONE DoubleRow matmul -> qr = q_un/4
                pd = pmisc.tile([KE, W], F32, tag="pmisc")
                nc.tensor.matmul(
                    pd[:], ddist[:].rearrange("p (t c) -> p t c", t=2),
                    zdist[:].rearrange("p (t n) -> p t n", t=2),
                    perf_mode=DR, start=True, stop=True)
                qr = qp.tile([KE, W], F16, tag="qr")
                nc.vector.reciprocal(qr[:], pd[:])

                # q hi/lo fp8 split (rescaled to q' = QS*q_un) for the
                # DoubleRow selector broadcast
                qhl = qp.tile([KE, 2 * W], F8, tag="qhl")
                nc.vector.tensor_scalar_mul(qhl[:, 0:W], qr[:], 4.0 * QS)
                nc.vector.scalar_tensor_tensor(
                    qhl[:, W:2 * W], qr[:], 4.0 * QS, qhl[:, 0:W],
                    op0=mybir.AluOpType.mult, op1=mybir.AluOpType.subtract)

                # sum_k q' -> rqs = 1/(QS * sum q) -> broadcast to 32 rows
                pqs = pmisc.tile([1, W], F32, tag="pmisc")
                nc.tensor.matmul(pqs[:], o16[:], qr[:], start=True, stop=True)
                rqs = qp.tile([1, W], F32, tag="rqs")
                nc.vector.reciprocal(rqs[:], pqs[:])
                prb_sb = qp.tile([32, W], F32, tag="prb_sb")
                nc.gpsimd.partition_broadcast(prb_sb[:], rqs[:])
                return dict(zt2=zt2, qr=qr, qhl=qhl, prb_sb=prb_sb, n0=n0)

            ectr = [0]

            def eh_relu(eh, pe_, force=None):
                # gpsimd cannot read PSUM: relus go on ACT/DVE only
                eng = force or epat[ectr[0] % len(epat)]
                ectr[0] += 1
                if eng == "A":
                    nc.scalar.activation(eh[:], pe_[:], AF.Relu, bias=0.0)
                else:
                    nc.vector.tensor_scalar_max(eh[:], pe_[:], 0.0)

            def back(st):
                zt2, qr, qhl, prb_sb, n0 = (st["zt2"], st["qr"], st["qhl"],
                                            st["prb_sb"], st["n0"])
                qhl3 = qhl[:].rearrange("p (t n) -> p t n", t=2)
                pp = ppred.tile([32, W], F32, tag="ppred")
                ncomb = NPAIR * 4
                if has_b2:
                    nc.tensor.matmul(pp[:], late["b2q"][:], qr[:],
                                     start=True, stop=False)

                zqs = {}

                def emit_qzq(j):
                    pqbt = pqb.tile([128, W], F32, tag="pqb")
                    nc.tensor.matmul(
                        pqbt[:],
                        late["e2p"][:, j * 256:(j + 1) * 256].rearrange(
                            "p (t c) -> p t c", t=2),
                        qhl3, perf_mode=DR, start=True, stop=True)
                    if not has_b1:
                        zq = zqp.tile([128, W], F16, tag="zq")
                        nc.vector.tensor_mul(zq[:], zt2[:], pqbt[:])
                        zqs[j] = zq
                    else:
                        zqs[j] = zt2

                def emit_w1(i):
                    j, hc, half = i // 4, (i // 2) % 2, i % 2
                    k = 2 * j + half
                    idx = k * 2 + hc
                    pe_ = pbig.tile([128, W], F32, tag="pbig")
                    nc.tensor.matmul(
                        pe_[:],
                        late["w1p"][64 * half:64 * (half + 1),
                            j * H_EXP + hc * 128: j * H_EXP + (hc + 1) * 128],
                        zqs[j][64 * half:64 * (half + 1), :],
                        start=True, stop=True,
                        tile_position=(64 * half, 0),
                    )
                    eh = ehp.tile([128, W], F16, tag="eh")
                    if not has_b1:
                        eh_relu(eh, pe_)
                    else:
                        ehr = ehp.tile([128, W], F16, tag="ehr")
                        nc.scalar.activation(
                            ehr[:], pe_[:], AF.Relu,
                            bias=late["b1c"][:, idx:idx + 1])
                        pqk = pqb.tile([128, W], F32, tag="pqb")
                        nc.tensor.matmul(
                            pqk[:],
                            late["e2s"][:, k * 256:(k + 1) * 256].rearrange(
                                "p (t c) -> p t c", t=2),
                            qhl3, perf_mode=DR, start=True, stop=True)
                        nc.vector.tensor_mul(eh[:], ehr[:], pqk[:])
                    return idx, eh

                def emit_w2(idx, eh, ci):
                    nc.tensor.matmul(
                        pp[:],
                        late["w2c"][:, idx * 32:(idx + 1) * 32],
                        eh[:],
                        start=(ci == 1 and not has_b2),
                        stop=(ci == ncomb),
                        skip_group_check=True,
                    )

                # software-pipelined expert loop: e2/zq for pair p lands EQ
                # W1-slots early; W2(i) trails W1(i) by LAG so the PE has
                # independent work while the relu drains.
                LAG, EQ = cfg["lag"], cfg["eq"]
                pend = {}
                ci = 0
                emit_qzq(0)
                for i in range(ncomb + LAG):
                    if i < ncomb:
                        nxt = i + EQ
                        if nxt % 4 == 0 and 0 < nxt // 4 < NPAIR:
                            emit_qzq(nxt // 4)
                        pend[i] = emit_w1(i)
                    if i >= LAG:
                        ci += 1
                        emit_w2(*pend.pop(i - LAG), ci)

                # scale by rqs (fused PSUM read + mul), transpose, store.
                # The last block's tail is split in halves so the post-PE
                # serial chain (stt -> transpose -> DMA) is shorter.
                nh = 2 if st.get("last") else 1
                hw_ = W // nh
                tm = trp.tile([32, W], F32, tag="tm")
                tr = trp.tile([32, W], F32, tag="tr")
                for h0 in range(nh):
                    cs = slice(h0 * hw_, (h0 + 1) * hw_)
                    nc.vector.scalar_tensor_tensor(
                        tm[:, cs], pp[:, cs], 0.0, prb_sb[:, cs],
                        op0=mybir.AluOpType.bypass, op1=mybir.AluOpType.mult)
                    nc.vector.transpose(tr[:, cs], tm[:, cs])
                    nc.sync.dma_start(
                        OUT[n0 + h0 * hw_:n0 + (h0 + 1) * hw_, :].rearrange(
                            "(b p) c -> p b c", p=32),
                        tr[:, cs].rearrange("p (b v) -> p b v", v=32)[
                            :, 0:hw_ // 32, 0:C],
                    )

            A = cfg["ahead"]
            for _rep in range(cfg["repeat"]):
                pre = pre_x if _rep == 0 else {}
                sts = [front(0, pre=pre.get(0))]
                if _rep == 0 and not late:
                    load_late_weights()
                for ib in range(1, min(A, NBLK)):
                    sts.append(front(ib, pre=pre.get(ib)))
                for ib in range(NBLK):
                    if ib + A < NBLK:
                        sts.append(front(ib + A, pre=pre.get(ib + A)))
                    if ib == NBLK - 1:
                        sts[ib]["last"] = True
                    back(sts[ib])
                sts.clear()

    nc.compile()
    return nc


def _prep(inputs):
    import ml_dtypes
    F8NP = ml_dtypes.float8_e4m3
    F16NP = np.float16

    f = lambda a: np.ascontiguousarray(np.asarray(a, dtype=np.float32))
    X, enc_W, enc_b = f(inputs["X"]), f(inputs["enc_W"]), f(inputs["enc_b"])
    z_W, z_b, mu = f(inputs["z_W"]), f(inputs["z_b"]), f(inputs["mu"])
    W1, b1, W2, b2 = f(inputs["W1"]), f(inputs["b1"]), f(inputs["W2"]), f(inputs["b2"])

    has_b1 = bool(np.any(b1))
    has_b2 = bool(np.any(b2))

    def hilo(a, s):
        hi = (a * s).astype(F8NP)
        lo = ((a * s) - hi.astype(np.float32)).astype(F8NP)
        return hi, lo

    XT = np.ascontiguousarray(X.T)                       # [D, N]
    xhi, xlo = hilo(XT, SX)

    # encoder weights packed for DoubleRow: [128][dcp=4][hc=4][two][128]
    whi, wlo = hilo(enc_W, SW)                           # [1024, 512]
    def encpack(w8):
        out = np.zeros((128, 4, 4, 2, 128), F8NP)
        w3 = w8.reshape(8, 128, 4, 128)                  # [dc][p][hc][c]
        for dcp in range(4):
            for t in range(2):
                out[:, dcp, :, t, :] = w3[2 * dcp + t]
        return out.reshape(128, 4096)
    com = {
        "WEH": encpack(whi),
        "WEL": encpack(wlo),
        "BENC": np.ascontiguousarray(enc_b.reshape(4, 128).T),
        "BZ": np.tile(z_b, 2).reshape(128, 1).copy(),
        "BZ4": (4.0 * z_b).reshape(NZ, 1).copy(),
        "O16": np.full((KE, 1), 4.0 * QS, F16NP),
    }
    # dist DoubleRow stationary [65, 2, 16] fp8:
    #   group0: rows 0:64 = -2 mu^T, row 64 = bias_hi     (vs ifmap 4z, 4.0)
    #   group1: rows 0:64 = 0.25,    row 64 = bias_lo     (vs ifmap 16z^2, 4.0)
    # -> PSUM = 4*(1 + |z|^2 - 2 z.mu + |mu|^2) = 4*(1+dist)
    bias = (1.0 + (mu.astype(np.float64) ** 2).sum(axis=1)).astype(np.float32)
    bias_hi = bias.astype(F8NP)
    bias_lo = (bias - bias_hi.astype(np.float32)).astype(F8NP)
    ddist = np.zeros((NZ + 1, 2, KE), F8NP)
    ddist[0:NZ, 0, :] = (-2.0 * mu.T).astype(F8NP)
    ddist[0:NZ, 1, :] = 0.25
    ddist[NZ, 0, :] = bias_hi
    ddist[NZ, 1, :] = bias_lo
    com["DDIST"] = ddist.reshape(NZ + 1, 2 * KE)

    wzd = np.zeros((128, 4, 128), np.float32)
    zw3 = z_W.reshape(4, 128, NZ)
    for hc in range(4):
        wzd[:, hc, 0:NZ] = zw3[hc]
        wzd[:, hc, NZ:128] = zw3[hc]
    com["WZ"] = wzd.reshape(128, 512).astype(F16NP)

    e2p = np.zeros((KE, NPAIR, 2, 128), F8NP)
    for j in range(NPAIR):
        sel = np.zeros((KE, 128), np.float32)
        sel[2 * j, 0:64] = 1.0
        sel[2 * j + 1, 64:128] = 1.0
        e2p[:, j, 0, :] = sel
        e2p[:, j, 1, :] = sel
    com["E2P"] = e2p.reshape(KE, NPAIR * 2 * 128)

    w1p = np.zeros((128, NPAIR * H_EXP), F16NP)
    for j in range(NPAIR):
        w1p[0:64, j * H_EXP:(j + 1) * H_EXP] = W1[2 * j]
        w1p[64:128, j * H_EXP:(j + 1) * H_EXP] = W1[2 * j + 1]
    com["W1P"] = w1p

    w2c = np.zeros((128, KE * 2 * 32), F16NP)
    for k in range(KE):
        for hc in range(2):
            w2c[:, (k * 2 + hc) * 32:(k * 2 + hc) * 32 + C] = \
                W2[k][hc * 128:(hc + 1) * 128, :]
    com["W2C"] = w2c

    if has_b2:
        b2q = np.zeros((KE, 32), F16NP)
        b2q[:, 0:C] = 4.0 * QS * b2
        com["B2Q"] = b2q
    if has_b1:
        b1c = np.zeros((128, KE * 2), np.float32)
        for k in range(KE):
            for hc in range(2):
                b1c[:, k * 2 + hc] = b1[k, hc * 128:(hc + 1) * 128]
        com["B1C"] = b1c
        e2s = np.zeros((KE, KE, 2, 128), F8NP)
        for k in range(KE):
            sel = np.zeros((KE, 128), np.float32)
            sel[k, :] = 1.0
            e2s[:, k, 0, :] = sel
            e2s[:, k, 1, :] = sel
        com["E2S"] = e2s.reshape(KE, KE * 2 * 128)

    # Per-core block-contiguous layout [NBLK][128][dc=8][NB]:
    # element [ib][p][dc][n] = X^T[dc*128+p, core*NS + ib*NB + n]
    def blockify(a8):  # [D, N] fp8 -> per-core [NBLK, 128, 8*NB]
        out = []
        for c in range(NCORES):
            v = a8[:, c * NS:(c + 1) * NS].reshape(8, 128, NBLK, NB)
            out.append(np.ascontiguousarray(
                v.transpose(2, 1, 0, 3).reshape(NBLK, 128, 8 * NB)))
        return out

    xhis, xlos = blockify(xhi), blockify(xlo)
    in_maps = []
    for c in range(NCORES):
        m = dict(com)
        m["XHI"] = xhis[c]
        m["XLO"] = xlos[c]
        in_maps.append(m)
    return in_maps, has_b1, has_b2


def kernel(**inputs) -> np.ndarray:
    global LAST_RESULTS
    from concourse.bass_utils import run_bass_kernel_spmd

    in_maps, has_b1, has_b2 = _prep(inputs)
    key = (has_b1, has_b2)
    if key not in _CACHE:
        _CACHE[key] = _build(has_b1, has_b2)
    nc = _CACHE[key]

    res = run_bass_kernel_spmd(nc, in_maps, list(range(NCORES)))
    LAST_RESULTS = res
    out = np.concatenate([res.results[c]["OUT"] for c in range(NCORES)], axis=0)
    return np.ascontiguousarray(out, dtype=np.float32)
